# revision 17
# baseline (speedup 1.0000x reference)
"""Bidirectional Mamba block kernel for 8 Trainium2 NeuronCores.

Sharding: core = (batch in 2) x (direction in 2) x (state-half in 2).
Each core processes the full sequence for one (batch, direction) with all
d_inner channels but 8 of the 16 SSM states; the two state-half partial
outputs are summed on the host (linear unshard). The bwd direction is
handled by a host-side time flip + feature-half swap so that all 8 cores
run one identical SPMD program.

Device pipeline per time chunk (TC columns):
  LayerNorm (PE ones-matmul stats, GPSIMD normalize)
  -> in_proj with the depthwise causal conv folded into the matmul
     (4 shifted weight taps, zero-padded at t<3)   [PE]
  -> SiLU evacuations fused into PSUM->SBUF copies [ACT]
  -> x_proj / dt_proj (+ softplus = ln(1+exp) fused in evac)
  -> selective scan: per 128-channel group, 8 per-state
     tensor_tensor_scan instructions on VectorE, chunk-chained
     via the `initial` operand reading a saved last-state column
  -> C-projection multiply + binary tree state reduction [DVE]
  -> gating (y + D*xc) * silu(z) [GPSIMD]
  -> out_proj + fp32 residual (+ output DMA, transposed layout)
"""

import sys

sys.path.insert(0, "/opt/trn_rl_repo")

import numpy as np
import ml_dtypes

import concourse.bacc as bacc
import concourse.mybir as mybir
import concourse.tile as tile
from concourse import bass_utils

F32 = mybir.dt.float32
BF16 = mybir.dt.bfloat16
AF = mybir.ActivationFunctionType
Alu = mybir.AluOpType
BF = ml_dtypes.bfloat16

EPS = 1e-5
D_CONV = 4
D_STATE = 16

# CoreSim does not implement the Silu activation table; for simulator
# validation we compose silu = x * sigmoid(x) instead (identical math).
SILU_VIA_SIGMOID = False


def default_cfg():
    return dict(T=4096, DM=1024, TC=512)


def derived(cfg):
    T, DM, TC = cfg["T"], cfg["DM"], cfg["TC"]
    d = dict(cfg)
    d["DH"] = DM // 2          # per-direction model dim
    d["DI"] = DM               # mamba inner dim (2 * DH)
    d["DTR"] = (d["DH"] + 15) // 16
    d["NSC"] = 8               # states per core (16 total, split 2-way)
    d["NCH"] = T // TC         # chunks
    d["NG"] = d["DI"] // 128   # 128-channel groups of d_inner
    d["NKF"] = d["DH"] // 128  # feature k-tiles (per-direction half)
    d["NGM"] = DM // 128       # feature groups for LN stats
    d["MO"] = d["DH"] // 128   # out_proj m-tiles
    return d


def _silu_evac(nc, sb, TC, out_ap, psum_ap, bias_col):
    """out = silu(psum + bias), PSUM -> SBUF bf16."""
    if not SILU_VIA_SIGMOID:
        nc.scalar.activation(out_ap, psum_ap, AF.Silu, bias=bias_col)
        return
    sg = sb.tile([psum_ap.shape[0], TC], BF16, tag="silu_sg", bufs=1)
    nc.scalar.activation(sg[:], psum_ap, AF.Sigmoid, bias=bias_col)
    xb = sb.tile([psum_ap.shape[0], TC], BF16, tag="silu_xb", bufs=1)
    nc.scalar.activation(xb[:], psum_ap, AF.Identity, bias=bias_col)
    nc.vector.tensor_tensor(out_ap, xb[:], sg[:], Alu.mult)


def build_nc(cfg):
    """Trace the single-core SPMD program. Returns (nc, derived-cfg)."""
    c = derived(cfg)
    T, TC, NCH = c["T"], c["TC"], c["NCH"]
    DM, DH, DI, DTR, NSC = c["DM"], c["DH"], c["DI"], c["DTR"], c["NSC"]
    NG, NKF, NGM, MO = c["NG"], c["NKF"], c["NGM"], c["MO"]

    nc = bacc.Bacc(
        "TRN2",
        target_bir_lowering=False,
        debug=False,
        enable_asserts=False,
        num_devices=8,
    )

    # ---- DRAM I/O ----------------------------------------------------------
    xT = nc.dram_tensor("xT", [DM, T], F32, kind="ExternalInput").ap()
    w_xc_T = nc.dram_tensor("w_xc_T", [4 * NKF * 128, DI], BF16, kind="ExternalInput").ap()
    w_z_T = nc.dram_tensor("w_z_T", [NKF * 128, DI], BF16, kind="ExternalInput").ap()
    w_xp_T = nc.dram_tensor("w_xp_T", [DI, DTR + 16], BF16, kind="ExternalInput").ap()
    w_dt_T = nc.dram_tensor("w_dt_T", [DTR, DI], BF16, kind="ExternalInput").ap()
    w_out_T = nc.dram_tensor("w_out_T", [DI, DH], BF16, kind="ExternalInput").ap()
    bias_xc = nc.dram_tensor("bias_xc", [DI, 1], F32, kind="ExternalInput").ap()
    bias_z = nc.dram_tensor("bias_z", [DI, 1], F32, kind="ExternalInput").ap()
    dt_bias = nc.dram_tensor("dt_bias", [DI, 1], F32, kind="ExternalInput").ap()
    A_cols = nc.dram_tensor("A_cols", [DI, NSC], F32, kind="ExternalInput").ap()
    D_vec = nc.dram_tensor("D_vec", [DI, 1], F32, kind="ExternalInput").ap()
    res_gate = nc.dram_tensor("res_gate", [128, 1], F32, kind="ExternalInput").ap()
    outT = nc.dram_tensor("outT", [DH, T], F32, kind="ExternalOutput").ap()

    with tile.TileContext(nc) as tc:
        with tc.tile_pool(name="wp", bufs=1) as wp, \
             tc.tile_pool(name="sb", bufs=1) as sb, \
             tc.tile_pool(name="dp", bufs=2, space="DRAM") as dp, \
             tc.tile_pool(name="ps", bufs=1, space="PSUM") as ps:

            # ---- resident weights -----------------------------------------
            w_xc_sb = wp.tile([128, 4 * NKF, DI], BF16)
            nc.sync.dma_start(w_xc_sb[:], w_xc_T.rearrange("(b k) m -> k b m", k=128))
            w_z_sb = wp.tile([128, NKF, DI], BF16)
            nc.sync.dma_start(w_z_sb[:], w_z_T.rearrange("(b k) m -> k b m", k=128))
            w_xp_sb = wp.tile([128, NG, DTR + 16], BF16)
            nc.sync.dma_start(w_xp_sb[:], w_xp_T.rearrange("(b k) m -> k b m", k=128))
            w_dt_sb = wp.tile([DTR, DI], BF16)
            nc.sync.dma_start(w_dt_sb[:], w_dt_T[:])
            w_out_sb = wp.tile([128, NG, DH], BF16)
            nc.sync.dma_start(w_out_sb[:], w_out_T.rearrange("(b k) m -> k b m", k=128))

            bias_xc_sb = wp.tile([128, NG, 1], F32)
            nc.sync.dma_start(bias_xc_sb[:], bias_xc.rearrange("(g k) o -> k g o", k=128))
            bias_z_sb = wp.tile([128, NG, 1], F32)
            nc.sync.dma_start(bias_z_sb[:], bias_z.rearrange("(g k) o -> k g o", k=128))
            dt_b_sb = wp.tile([128, NG, 1], F32)
            nc.sync.dma_start(dt_b_sb[:], dt_bias.rearrange("(g k) o -> k g o", k=128))
            A_sb = wp.tile([128, NG, NSC], F32)
            nc.sync.dma_start(A_sb[:], A_cols.rearrange("(g k) n -> k g n", k=128))
            D_sb = wp.tile([128, NG, 1], F32)
            nc.sync.dma_start(D_sb[:], D_vec.rearrange("(g k) o -> k g o", k=128))
            rgate_sb = wp.tile([128, 1], F32)
            nc.sync.dma_start(rgate_sb[:], res_gate[:])

            ones_col = wp.tile([128, 1], BF16)
            nc.vector.memset(ones_col[:], 1.0)
            eps_col = wp.tile([1, 1], F32)
            nc.vector.memset(eps_col[:], EPS)
            one_col = wp.tile([128, 1], F32)
            nc.vector.memset(one_col[:], 1.0)

            hlast_prev = None
            xn_prev = None

            for ci in range(NCH):
                ts = slice(ci * TC, (ci + 1) * TC)

                # ---- load x chunk: bf16 copy (all rows) + fp32 rows for
                # the residual (first DH rows only)
                x_bf = sb.tile([128, NGM, TC], BF16, tag="x_bf", bufs=1)
                nc.gpsimd.dma_start(
                    x_bf[:], xT[:, ts].rearrange("(g k) t -> k g t", k=128)
                )
                x_res = sb.tile([128, MO, TC], F32, tag="x_res", bufs=1)
                nc.sync.dma_start(
                    x_res[:], xT[0:DH, ts].rearrange("(g k) t -> k g t", k=128)
                )

                # ---- LayerNorm stats --------------------------------------
                mu_ps = ps.tile([1, TC], F32, tag="mu_ps", bufs=1)
                sq_ps = ps.tile([1, TC], F32, tag="sq_ps", bufs=1)
                for g in range(NGM):
                    nc.tensor.matmul(
                        mu_ps[:], ones_col[:], x_bf[:, g, :],
                        start=(g == 0), stop=(g == NGM - 1),
                    )
                for g in range(NGM):
                    xsq = sb.tile([128, TC], BF16, tag="xsq", bufs=2)
                    nc.scalar.activation(xsq[:], x_bf[:, g, :], AF.Square)
                    nc.tensor.matmul(
                        sq_ps[:], ones_col[:], xsq[:],
                        start=(g == 0), stop=(g == NGM - 1),
                    )
                mu_row = sb.tile([1, TC], F32, tag="mu_row", bufs=1)
                nc.scalar.mul(mu_row[:], mu_ps[:], 1.0 / DM)
                msq_row = sb.tile([1, TC], F32, tag="msq_row", bufs=1)
                nc.scalar.mul(msq_row[:], sq_ps[:], 1.0 / DM)
                mu2_row = sb.tile([1, TC], F32, tag="mu2_row", bufs=1)
                nc.vector.tensor_tensor(mu2_row[:], mu_row[:], mu_row[:], Alu.mult)
                var_row = sb.tile([1, TC], F32, tag="var_row", bufs=1)
                nc.vector.tensor_tensor(var_row[:], msq_row[:], mu2_row[:], Alu.subtract)
                # rstd = exp(-0.5 * ln(var + eps)) -- stays in the ln/exp table set
                lv_row = sb.tile([1, TC], F32, tag="lv_row", bufs=1)
                nc.scalar.activation(lv_row[:], var_row[:], AF.Ln, bias=eps_col[:])
                rstd_row = sb.tile([1, TC], F32, tag="rstd_row", bufs=1)
                nc.scalar.activation(rstd_row[:], lv_row[:], AF.Exp, scale=-0.5)
                nmr_row = sb.tile([1, TC], F32, tag="nmr_row", bufs=1)
                nc.vector.scalar_tensor_tensor(
                    nmr_row[:], mu_row[:], -1.0, rstd_row[:], Alu.mult, Alu.mult
                )
                rn_dram = dp.tile([2, TC], F32, tag="rn_dram", bufs=2)
                nc.gpsimd.dma_start(rn_dram[0:1, :], rstd_row[:])
                nc.gpsimd.dma_start(rn_dram[1:2, :], nmr_row[:])
                rstd_bc = sb.tile([128, TC], F32, tag="rstd_bc", bufs=1)
                nc.gpsimd.dma_start(rstd_bc[:], rn_dram[0, :].partition_broadcast(128))
                nmr_bc = sb.tile([128, TC], F32, tag="nmr_bc", bufs=1)
                nc.gpsimd.dma_start(nmr_bc[:], rn_dram[1, :].partition_broadcast(128))

                # ---- normalize (only the direction's feature half) --------
                xn = sb.tile([128, NKF, TC], BF16, tag="xn", bufs=2)
                for g in range(NKF):
                    lntmp = sb.tile([128, TC], BF16, tag="lntmp", bufs=1)
                    nc.gpsimd.tensor_tensor(lntmp[:], x_bf[:, g, :], rstd_bc[:], Alu.mult)
                    nc.gpsimd.tensor_tensor(xn[:, g, :], lntmp[:], nmr_bc[:], Alu.add)

                # ---- in_proj xc-half with conv folded in ------------------
                xc_t = sb.tile([128, NG, TC], BF16, tag="xc_t", bufs=1)
                for m in range(NG):
                    xz_ps = ps.tile([128, TC], F32, tag="xz_ps", bufs=2)
                    mm = []  # (lhsT, rhs, out)
                    msl = slice(m * 128, (m + 1) * 128)
                    for kk in range(NKF):  # tap j=3: unshifted, covers all cols
                        mm.append((w_xc_sb[:, 3 * NKF + kk, msl],
                                   xn[:, kk, :], xz_ps[:]))
                    for j in range(3):     # taps j=0..2, shift s=3-j
                        s = 3 - j
                        for kk in range(NKF):
                            mm.append((w_xc_sb[:, j * NKF + kk, msl],
                                       xn[:, kk, 0:TC - s], xz_ps[:, s:TC]))
                            if ci > 0:
                                mm.append((w_xc_sb[:, j * NKF + kk, msl],
                                           xn_prev[:, kk, TC - s:TC], xz_ps[:, 0:s]))
                    for i, (l, r, o) in enumerate(mm):
                        nc.tensor.matmul(o, l, r, start=(i == 0),
                                         stop=(i == len(mm) - 1))
                    _silu_evac(nc, sb, TC, xc_t[:, m, :], xz_ps[:],
                               bias_xc_sb[:, m, :])

                # ---- in_proj z-half + SiLU --------------------------------
                gz = sb.tile([128, NG, TC], BF16, tag="gz", bufs=1)
                for m in range(NG):
                    z_ps = ps.tile([128, TC], F32, tag="xz_ps", bufs=2)
                    for kk in range(NKF):
                        nc.tensor.matmul(z_ps[:], w_z_sb[:, kk, m * 128:(m + 1) * 128],
                                         xn[:, kk, :],
                                         start=(kk == 0), stop=(kk == NKF - 1))
                    _silu_evac(nc, sb, TC, gz[:, m, :], z_ps[:],
                               bias_z_sb[:, m, :])

                # ---- x_proj ----------------------------------------------
                xd_ps = ps.tile([DTR + 16, TC], F32, tag="xd_ps", bufs=1)
                for g in range(NG):
                    nc.tensor.matmul(xd_ps[:], w_xp_sb[:, g, :], xc_t[:, g, :],
                                     start=(g == 0), stop=(g == NG - 1))
                x_dbl = sb.tile([DTR + 16, TC], BF16, tag="x_dbl", bufs=2)
                nc.scalar.activation(x_dbl[:], xd_ps[:], AF.Copy)

                # ---- dt_proj; dt = softplus(dt_pre + b) = ln(1+exp(.)) ----
                dt_t = sb.tile([128, NG, TC], BF16, tag="dt_t", bufs=1)
                for m in range(NG):
                    dt_ps = ps.tile([128, TC], F32, tag="dt_ps", bufs=1)
                    nc.tensor.matmul(dt_ps[:], w_dt_sb[:, m * 128:(m + 1) * 128],
                                     x_dbl[0:DTR, :], start=True, stop=True)
                    edt = sb.tile([128, TC], BF16, tag="edt", bufs=1)
                    nc.scalar.activation(edt[:], dt_ps[:], AF.Exp,
                                         bias=dt_b_sb[:, m, :])
                    nc.scalar.activation(dt_t[:, m, :], edt[:], AF.Ln,
                                         bias=one_col[:])

                # ---- dt * u ----------------------------------------------
                dtu = sb.tile([128, NG, TC], BF16, tag="dtu", bufs=1)
                for g in range(NG):
                    nc.gpsimd.tensor_tensor(dtu[:, g, :], dt_t[:, g, :],
                                            xc_t[:, g, :], Alu.mult)

                # ---- broadcast B and C rows to all partitions ------------
                bc_dram = dp.tile([2 * NSC, TC], BF16, tag="bc_dram", bufs=2)
                nc.gpsimd.dma_start(bc_dram[:], x_dbl[DTR:DTR + 2 * NSC, :])
                B_bc = sb.tile([128, NSC, TC], BF16, tag="B_bc", bufs=1)
                nc.gpsimd.dma_start(
                    B_bc[:], bc_dram[0:NSC, :].partition_broadcast(128))
                C_bc = sb.tile([128, NSC, TC], BF16, tag="C_bc", bufs=1)
                nc.gpsimd.dma_start(
                    C_bc[:], bc_dram[NSC:2 * NSC, :].partition_broadcast(128))

                # ---- selective scan per channel group --------------------
                hlast = sb.tile([128, NG, NSC], BF16, tag="hlast", bufs=2)
                ygated = sb.tile([128, NG, TC], BF16, tag="ygated", bufs=1)
                for g in range(NG):
                    h_t = sb.tile([128, NSC, TC], BF16, tag="h_t", bufs=1)
                    for n in range(NSC):
                        dA = sb.tile([128, TC], BF16, tag="dA", bufs=2)
                        nc.scalar.activation(dA[:], dt_t[:, g, :], AF.Exp,
                                             scale=A_sb[:, g, n:n + 1])
                        wv = sb.tile([128, TC], BF16, tag="wv", bufs=1)
                        nc.vector.tensor_tensor(wv[:], dtu[:, g, :],
                                                B_bc[:, n, :], Alu.mult)
                        init = 0.0 if ci == 0 else hlast_prev[:, g, n:n + 1]
                        nc.vector.tensor_tensor_scan(
                            h_t[:, n, :], dA[:], wv[:], init,
                            Alu.mult, Alu.add,
                        )
                    # save last columns for the next chunk's initial state
                    nc.vector.tensor_copy(hlast[:, g, :], h_t[:, :, TC - 1:TC])
                    hC = sb.tile([128, NSC, TC], BF16, tag="hC", bufs=1)
                    nc.vector.tensor_tensor(hC[:], h_t[:], C_bc[:], Alu.mult)
                    t1 = sb.tile([128, 4, TC], BF16, tag="t1", bufs=1)
                    nc.vector.tensor_tensor(t1[:], hC[:, 0:4, :], hC[:, 4:8, :], Alu.add)
                    t2 = sb.tile([128, 2, TC], BF16, tag="t2", bufs=1)
                    nc.vector.tensor_tensor(t2[:], t1[:, 0:2, :], t1[:, 2:4, :], Alu.add)
                    ysum = sb.tile([128, TC], BF16, tag="ysum", bufs=2)
                    nc.vector.tensor_tensor(ysum[:], t2[:, 0, :], t2[:, 1, :], Alu.add)
                    # gating: (y + D*xc) * silu(z)
                    tg = sb.tile([128, TC], BF16, tag="tg", bufs=1)
                    nc.vector.scalar_tensor_tensor(
                        tg[:], xc_t[:, g, :], D_sb[:, g, :], ysum[:],
                        Alu.mult, Alu.add,
                    )
                    nc.gpsimd.tensor_tensor(ygated[:, g, :], tg[:], gz[:, g, :],
                                            Alu.mult)
                hlast_prev = hlast

                # ---- out_proj + residual ---------------------------------
                for mo in range(MO):
                    o_ps = ps.tile([128, TC], F32, tag="o_ps", bufs=2)
                    for g in range(NG):
                        nc.tensor.matmul(
                            o_ps[:], w_out_sb[:, g, mo * 128:(mo + 1) * 128],
                            ygated[:, g, :], start=(g == 0), stop=(g == NG - 1),
                        )
                    out_sb = sb.tile([128, TC], F32, tag="out_sb", bufs=2)
                    nc.vector.scalar_tensor_tensor(
                        out_sb[:], x_res[:, mo, :], rgate_sb[:, 0:1], o_ps[:],
                        Alu.mult, Alu.add,
                    )
                    nc.sync.dma_start(outT[mo * 128:(mo + 1) * 128, ts], out_sb[:])

                xn_prev = xn

    nc.compile()
    return nc, c


# ---------------------------------------------------------------------------
# Host-side sharding
# ---------------------------------------------------------------------------

def host_shard(inputs, cfg):
    """Build the 8 per-core input maps from the full problem inputs."""
    c = derived(cfg)
    T, DM, DH, DI, DTR, NSC = c["T"], c["DM"], c["DH"], c["DI"], c["DTR"], c["NSC"]
    NKF = c["NKF"]

    x = np.asarray(inputs["x"], np.float32)          # (B, T, DM)
    norm_w = np.asarray(inputs["norm_w"], np.float32)
    norm_b = np.asarray(inputs["norm_b"], np.float32)

    in_maps = []
    for b in range(2):
        for d in range(2):
            pre = "fwd" if d == 0 else "bwd"
            if d == 0:
                xb = x[b]
                nw, nb = norm_w, norm_b
            else:
                xb = x[b][::-1]
                xb = np.concatenate([xb[:, DH:], xb[:, :DH]], axis=1)
                nw = np.concatenate([norm_w[DH:], norm_w[:DH]])
                nb = np.concatenate([norm_b[DH:], norm_b[:DH]])
            xT = np.ascontiguousarray(xb.T, dtype=np.float32)

            W = np.asarray(inputs[pre + "_in_proj_w"], np.float32)   # (2DI, DH)
            conv_w = np.asarray(inputs[pre + "_conv_w"], np.float32)[:, 0, :]
            conv_b = np.asarray(inputs[pre + "_conv_b"], np.float32)
            xp = np.asarray(inputs[pre + "_x_proj_w"], np.float32)
            wdt = np.asarray(inputs[pre + "_dt_proj_w"], np.float32)
            dtb = np.asarray(inputs[pre + "_dt_proj_b"], np.float32)
            A_log = np.asarray(inputs[pre + "_A_log"], np.float32)
            Dv = np.asarray(inputs[pre + "_D"], np.float32)
            wout = np.asarray(inputs[pre + "_out_proj_w"], np.float32)

            nwh, nbh = nw[:DH], nb[:DH]
            W_eff = W * nwh[None, :]
            bias_in = W @ nbh                                        # (2DI,)
            W_xc, W_z = W_eff[:DI], W_eff[DI:]

            blocks = []
            for j in range(4):
                scaled = conv_w[:, j:j + 1].T * W_xc.T               # (DH, DI)
                for kk in range(NKF):
                    blocks.append(scaled[kk * 128:(kk + 1) * 128, :])
            w_xc_T = np.ascontiguousarray(np.concatenate(blocks, 0)).astype(BF)
            bias_xc = (conv_b + bias_in[:DI] * conv_w.sum(1)).reshape(DI, 1)
            w_z_T = np.ascontiguousarray(W_z.T).astype(BF)
            bias_z = bias_in[DI:].reshape(DI, 1).astype(np.float32)

            base = dict(
                xT=xT,
                w_xc_T=w_xc_T,
                w_z_T=w_z_T,
                w_dt_T=np.ascontiguousarray(wdt.T).astype(BF),
                w_out_T=np.ascontiguousarray(wout.T).astype(BF),
                bias_xc=bias_xc.astype(np.float32),
                bias_z=bias_z,
                dt_bias=dtb.reshape(DI, 1).astype(np.float32),
            )
            for nh in range(2):
                sel = np.concatenate([
                    xp[:DTR],
                    xp[DTR + NSC * nh: DTR + NSC * nh + NSC],
                    xp[DTR + D_STATE + NSC * nh: DTR + D_STATE + NSC * nh + NSC],
                ], axis=0)                                           # (DTR+16, DI)
                m = dict(base)
                m["w_xp_T"] = np.ascontiguousarray(sel.T).astype(BF)
                m["A_cols"] = np.ascontiguousarray(
                    -np.exp(A_log[:, NSC * nh: NSC * nh + NSC])).astype(np.float32)
                m["D_vec"] = (Dv if nh == 0 else np.zeros_like(Dv)).reshape(DI, 1).astype(np.float32)
                m["res_gate"] = np.full((128, 1), 1.0 if nh == 0 else 0.0, np.float32)
                in_maps.append(m)
    return in_maps


def host_unshard(results, cfg):
    c = derived(cfg)
    T, DM, DH = c["T"], c["DM"], c["DH"]
    out = np.empty((2, T, DM), np.float32)
    for b in range(2):
        for d in range(2):
            o = results[b * 4 + d * 2 + 0]["outT"] + results[b * 4 + d * 2 + 1]["outT"]
            oT = o.T                                   # (T, DH)
            if d == 1:
                oT = oT[::-1]
            out[b, :, d * DH:(d + 1) * DH] = oT
    return out


_CACHE = {}


def _get_nc(cfg_key):
    if cfg_key not in _CACHE:
        cfg = dict(T=cfg_key[0], DM=cfg_key[1], TC=cfg_key[2])
        _CACHE[cfg_key] = build_nc(cfg)
    return _CACHE[cfg_key]


def kernel(**inputs):
    cfg = default_cfg()
    nc, _ = _get_nc((cfg["T"], cfg["DM"], cfg["TC"]))
    in_maps = host_shard(inputs, cfg)
    res = bass_utils.run_bass_kernel_spmd(nc, in_maps, core_ids=list(range(8)))
    return host_unshard(res.results, cfg)


# revision 18
# speedup vs baseline: 66.7917x; 66.7917x over previous
"""Bidirectional Mamba block kernel for 8 Trainium2 NeuronCores.

Sharding: core = (batch in 2) x (direction in 2) x (state-half in 2).
Each core processes the full sequence for one (batch, direction) with all
d_inner channels but 8 of the 16 SSM states; the two state-half partial
outputs are summed on the host (linear unshard). The bwd direction is
handled by a host-side time flip + feature-half swap so that all 8 cores
run one identical SPMD program.

Device pipeline per time chunk (TC columns):
  LayerNorm (PE ones-matmul stats, GPSIMD normalize)
  -> in_proj with the depthwise causal conv folded into the matmul
     (4 shifted weight taps, zero-padded at t<3)   [PE]
  -> SiLU evacuations fused into PSUM->SBUF copies [ACT]
  -> x_proj / dt_proj (+ softplus = ln(1+exp) fused in evac)
  -> selective scan: per 128-channel group, 8 per-state
     tensor_tensor_scan instructions on VectorE, chunk-chained
     via the `initial` operand reading a saved last-state column
  -> C-projection multiply + binary tree state reduction [DVE]
  -> gating (y + D*xc) * silu(z) [GPSIMD]
  -> out_proj + fp32 residual (+ output DMA, transposed layout)
"""

import sys

sys.path.insert(0, "/opt/trn_rl_repo")

import numpy as np
import ml_dtypes

import concourse.bacc as bacc
import concourse.mybir as mybir
import concourse.tile as tile
from concourse import bass_utils

F32 = mybir.dt.float32
BF16 = mybir.dt.bfloat16
AF = mybir.ActivationFunctionType
Alu = mybir.AluOpType
BF = ml_dtypes.bfloat16

EPS = 1e-5
D_CONV = 4
D_STATE = 16

# CoreSim does not implement the Silu activation table; for simulator
# validation we compose silu = x * sigmoid(x) instead (identical math).
SILU_VIA_SIGMOID = False


def default_cfg():
    return dict(T=4096, DM=1024, TC=512)


def derived(cfg):
    T, DM, TC = cfg["T"], cfg["DM"], cfg["TC"]
    d = dict(cfg)
    d["DH"] = DM // 2          # per-direction model dim
    d["DI"] = DM               # mamba inner dim (2 * DH)
    d["DTR"] = (d["DH"] + 15) // 16
    d["NSC"] = 8               # states per core (16 total, split 2-way)
    d["NCH"] = T // TC         # chunks
    d["NG"] = d["DI"] // 128   # 128-channel groups of d_inner
    d["NKF"] = d["DH"] // 128  # feature k-tiles (per-direction half)
    d["NGM"] = DM // 128       # feature groups for LN stats
    d["MO"] = d["DH"] // 128   # out_proj m-tiles
    return d


def _silu_evac(nc, sb, TC, out_ap, psum_ap, bias_col):
    """out = silu(psum + bias), PSUM -> SBUF bf16."""
    if not SILU_VIA_SIGMOID:
        nc.scalar.activation(out_ap, psum_ap, AF.Silu, bias=bias_col)
        return
    sg = sb.tile([psum_ap.shape[0], TC], BF16, tag="silu_sg", bufs=1)
    nc.scalar.activation(sg[:], psum_ap, AF.Sigmoid, bias=bias_col)
    xb = sb.tile([psum_ap.shape[0], TC], BF16, tag="silu_xb", bufs=1)
    nc.scalar.activation(xb[:], psum_ap, AF.Identity, bias=bias_col)
    nc.vector.tensor_tensor(out_ap, xb[:], sg[:], Alu.mult)


def build_nc(cfg):
    """Trace the single-core SPMD program. Returns (nc, derived-cfg)."""
    c = derived(cfg)
    T, TC, NCH = c["T"], c["TC"], c["NCH"]
    DM, DH, DI, DTR, NSC = c["DM"], c["DH"], c["DI"], c["DTR"], c["NSC"]
    NG, NKF, NGM, MO = c["NG"], c["NKF"], c["NGM"], c["MO"]

    nc = bacc.Bacc(
        "TRN2",
        target_bir_lowering=False,
        debug=False,
        enable_asserts=False,
        num_devices=8,
    )

    # ---- DRAM I/O ----------------------------------------------------------
    xT = nc.dram_tensor("xT", [DM, T], F32, kind="ExternalInput").ap()
    w_xc_T = nc.dram_tensor("w_xc_T", [4 * NKF * 128, DI], BF16, kind="ExternalInput").ap()
    w_z_T = nc.dram_tensor("w_z_T", [NKF * 128, DI], BF16, kind="ExternalInput").ap()
    w_xp_T = nc.dram_tensor("w_xp_T", [DI, DTR + 16], BF16, kind="ExternalInput").ap()
    w_dt_T = nc.dram_tensor("w_dt_T", [DTR, DI], BF16, kind="ExternalInput").ap()
    w_out_T = nc.dram_tensor("w_out_T", [DI, DH], BF16, kind="ExternalInput").ap()
    bias_xc = nc.dram_tensor("bias_xc", [DI, 1], F32, kind="ExternalInput").ap()
    bias_z = nc.dram_tensor("bias_z", [DI, 1], F32, kind="ExternalInput").ap()
    dt_bias = nc.dram_tensor("dt_bias", [DI, 1], F32, kind="ExternalInput").ap()
    A_cols = nc.dram_tensor("A_cols", [DI, NSC], F32, kind="ExternalInput").ap()
    D_vec = nc.dram_tensor("D_vec", [DI, 1], F32, kind="ExternalInput").ap()
    res_gate = nc.dram_tensor("res_gate", [128, 1], F32, kind="ExternalInput").ap()
    outT = nc.dram_tensor("outT", [DH, T], F32, kind="ExternalOutput").ap()

    with tile.TileContext(nc) as tc:
        with tc.tile_pool(name="wp", bufs=1) as wp, \
             tc.tile_pool(name="sb", bufs=1) as sb, \
             tc.tile_pool(name="dp", bufs=2, space="DRAM") as dp, \
             tc.tile_pool(name="ps", bufs=1, space="PSUM") as ps:

            # ---- resident weights -----------------------------------------
            w_xc_sb = wp.tile([128, 4 * NKF, DI], BF16)
            nc.sync.dma_start(w_xc_sb[:], w_xc_T.rearrange("(b k) m -> k b m", k=128))
            w_z_sb = wp.tile([128, NKF, DI], BF16)
            nc.sync.dma_start(w_z_sb[:], w_z_T.rearrange("(b k) m -> k b m", k=128))
            w_xp_sb = wp.tile([128, NG, DTR + 16], BF16)
            nc.sync.dma_start(w_xp_sb[:], w_xp_T.rearrange("(b k) m -> k b m", k=128))
            w_dt_sb = wp.tile([DTR, DI], BF16)
            nc.sync.dma_start(w_dt_sb[:], w_dt_T[:])
            w_out_sb = wp.tile([128, NG, DH], BF16)
            nc.sync.dma_start(w_out_sb[:], w_out_T.rearrange("(b k) m -> k b m", k=128))

            bias_xc_sb = wp.tile([128, NG, 1], F32)
            nc.sync.dma_start(bias_xc_sb[:], bias_xc.rearrange("(g k) o -> k g o", k=128))
            bias_z_sb = wp.tile([128, NG, 1], F32)
            nc.sync.dma_start(bias_z_sb[:], bias_z.rearrange("(g k) o -> k g o", k=128))
            dt_b_sb = wp.tile([128, NG, 1], F32)
            nc.sync.dma_start(dt_b_sb[:], dt_bias.rearrange("(g k) o -> k g o", k=128))
            A_sb = wp.tile([128, NG, NSC], F32)
            nc.sync.dma_start(A_sb[:], A_cols.rearrange("(g k) n -> k g n", k=128))
            D_sb = wp.tile([128, NG, 1], F32)
            nc.sync.dma_start(D_sb[:], D_vec.rearrange("(g k) o -> k g o", k=128))
            rgate_sb = wp.tile([128, 1], F32)
            nc.sync.dma_start(rgate_sb[:], res_gate[:])

            ones_col = wp.tile([128, 1], BF16)
            nc.vector.memset(ones_col[:], 1.0)
            eps_col = wp.tile([1, 1], F32)
            nc.vector.memset(eps_col[:], EPS)
            one_col = wp.tile([128, 1], F32)
            nc.vector.memset(one_col[:], 1.0)

            hlast_prev = None
            xn_prev = None

            for ci in range(NCH):
                ts = slice(ci * TC, (ci + 1) * TC)

                # ---- load x chunk: bf16 copy (all rows) + fp32 rows for
                # the residual (first DH rows only)
                x_bf = sb.tile([128, NGM, TC], BF16, tag="x_bf", bufs=1)
                nc.gpsimd.dma_start(
                    x_bf[:], xT[:, ts].rearrange("(g k) t -> k g t", k=128)
                )
                x_res = sb.tile([128, MO, TC], F32, tag="x_res", bufs=1)
                nc.sync.dma_start(
                    x_res[:], xT[0:DH, ts].rearrange("(g k) t -> k g t", k=128)
                )

                # ---- LayerNorm stats --------------------------------------
                mu_ps = ps.tile([1, TC], F32, tag="mu_ps", bufs=1)
                sq_ps = ps.tile([1, TC], F32, tag="sq_ps", bufs=1)
                for g in range(NGM):
                    nc.tensor.matmul(
                        mu_ps[:], ones_col[:], x_bf[:, g, :],
                        start=(g == 0), stop=(g == NGM - 1),
                    )
                for g in range(NGM):
                    xsq = sb.tile([128, TC], BF16, tag="xsq", bufs=2)
                    nc.scalar.activation(xsq[:], x_bf[:, g, :], AF.Square)
                    nc.tensor.matmul(
                        sq_ps[:], ones_col[:], xsq[:],
                        start=(g == 0), stop=(g == NGM - 1),
                    )
                mu_row = sb.tile([1, TC], F32, tag="mu_row", bufs=1)
                nc.scalar.mul(mu_row[:], mu_ps[:], 1.0 / DM)
                msq_row = sb.tile([1, TC], F32, tag="msq_row", bufs=1)
                nc.scalar.mul(msq_row[:], sq_ps[:], 1.0 / DM)
                mu2_row = sb.tile([1, TC], F32, tag="mu2_row", bufs=1)
                nc.vector.tensor_tensor(mu2_row[:], mu_row[:], mu_row[:], Alu.mult)
                var_row = sb.tile([1, TC], F32, tag="var_row", bufs=1)
                nc.vector.tensor_tensor(var_row[:], msq_row[:], mu2_row[:], Alu.subtract)
                # rstd = exp(-0.5 * ln(var + eps)) -- stays in the ln/exp table set
                lv_row = sb.tile([1, TC], F32, tag="lv_row", bufs=1)
                nc.scalar.activation(lv_row[:], var_row[:], AF.Ln, bias=eps_col[:])
                rstd_row = sb.tile([1, TC], F32, tag="rstd_row", bufs=1)
                nc.scalar.activation(rstd_row[:], lv_row[:], AF.Exp, scale=-0.5)
                nmr_row = sb.tile([1, TC], F32, tag="nmr_row", bufs=1)
                nc.vector.scalar_tensor_tensor(
                    nmr_row[:], mu_row[:], -1.0, rstd_row[:], Alu.mult, Alu.mult
                )
                rn_dram = dp.tile([2, TC], F32, tag="rn_dram", bufs=2)
                nc.gpsimd.dma_start(rn_dram[0:1, :], rstd_row[:])
                nc.gpsimd.dma_start(rn_dram[1:2, :], nmr_row[:])
                rstd_bc = sb.tile([128, TC], F32, tag="rstd_bc", bufs=1)
                nc.gpsimd.dma_start(rstd_bc[:], rn_dram[0, :].partition_broadcast(128))
                nmr_bc = sb.tile([128, TC], F32, tag="nmr_bc", bufs=1)
                nc.gpsimd.dma_start(nmr_bc[:], rn_dram[1, :].partition_broadcast(128))

                # ---- normalize (only the direction's feature half) --------
                xn = sb.tile([128, NKF, TC + 4], BF16, tag="xn", bufs=2)
                if ci == 0:
                    nc.vector.memset(xn[:, :, 0:4], 0.0)
                else:
                    nc.vector.tensor_copy(xn[:, :, 0:4],
                                          xn_prev[:, :, TC:TC + 4])
                for g in range(NKF):
                    lntmp = sb.tile([128, TC], BF16, tag="lntmp", bufs=1)
                    nc.gpsimd.tensor_tensor(lntmp[:], x_bf[:, g, :], rstd_bc[:], Alu.mult)
                    nc.gpsimd.tensor_tensor(xn[:, g, 4:TC + 4], lntmp[:], nmr_bc[:], Alu.add)

                # ---- in_proj xc-half with conv folded in ------------------
                xc_t = sb.tile([128, NG, TC], BF16, tag="xc_t", bufs=1)
                for m in range(NG):
                    xz_ps = ps.tile([128, TC], F32, tag="xz_ps", bufs=2)
                    mm = []  # (lhsT, rhs)
                    msl = slice(m * 128, (m + 1) * 128)
                    for j in range(4):     # tap j reads window starting at j+1
                        for kk in range(NKF):
                            mm.append((w_xc_sb[:, j * NKF + kk, msl],
                                       xn[:, kk, j + 1:j + 1 + TC]))
                    for i, (l, r) in enumerate(mm):
                        nc.tensor.matmul(xz_ps[:], l, r, start=(i == 0),
                                         stop=(i == len(mm) - 1))
                    _silu_evac(nc, sb, TC, xc_t[:, m, :], xz_ps[:],
                               bias_xc_sb[:, m, :])

                # ---- in_proj z-half + SiLU --------------------------------
                gz = sb.tile([128, NG, TC], BF16, tag="gz", bufs=1)
                for m in range(NG):
                    z_ps = ps.tile([128, TC], F32, tag="xz_ps", bufs=2)
                    for kk in range(NKF):
                        nc.tensor.matmul(z_ps[:], w_z_sb[:, kk, m * 128:(m + 1) * 128],
                                         xn[:, kk, 4:TC + 4],
                                         start=(kk == 0), stop=(kk == NKF - 1))
                    _silu_evac(nc, sb, TC, gz[:, m, :], z_ps[:],
                               bias_z_sb[:, m, :])

                # ---- x_proj ----------------------------------------------
                xd_ps = ps.tile([DTR + 16, TC], F32, tag="xd_ps", bufs=1)
                for g in range(NG):
                    nc.tensor.matmul(xd_ps[:], w_xp_sb[:, g, :], xc_t[:, g, :],
                                     start=(g == 0), stop=(g == NG - 1))
                x_dbl = sb.tile([DTR + 16, TC], BF16, tag="x_dbl", bufs=2)
                nc.scalar.activation(x_dbl[:], xd_ps[:], AF.Copy)

                # ---- dt_proj; dt = softplus(dt_pre + b) = ln(1+exp(.)) ----
                dt_t = sb.tile([128, NG, TC], BF16, tag="dt_t", bufs=1)
                for m in range(NG):
                    dt_ps = ps.tile([128, TC], F32, tag="dt_ps", bufs=1)
                    nc.tensor.matmul(dt_ps[:], w_dt_sb[:, m * 128:(m + 1) * 128],
                                     x_dbl[0:DTR, :], start=True, stop=True)
                    edt = sb.tile([128, TC], BF16, tag="edt", bufs=1)
                    nc.scalar.activation(edt[:], dt_ps[:], AF.Exp,
                                         bias=dt_b_sb[:, m, :])
                    nc.scalar.activation(dt_t[:, m, :], edt[:], AF.Ln,
                                         bias=one_col[:])

                # ---- dt * u ----------------------------------------------
                dtu = sb.tile([128, NG, TC], BF16, tag="dtu", bufs=1)
                for g in range(NG):
                    nc.gpsimd.tensor_tensor(dtu[:, g, :], dt_t[:, g, :],
                                            xc_t[:, g, :], Alu.mult)

                # ---- broadcast B and C rows to all partitions ------------
                bc_dram = dp.tile([2 * NSC, TC], BF16, tag="bc_dram", bufs=2)
                nc.gpsimd.dma_start(bc_dram[:], x_dbl[DTR:DTR + 2 * NSC, :])
                B_bc = sb.tile([128, NSC, TC], BF16, tag="B_bc", bufs=1)
                nc.gpsimd.dma_start(
                    B_bc[:], bc_dram[0:NSC, :].partition_broadcast(128))
                C_bc = sb.tile([128, NSC, TC], BF16, tag="C_bc", bufs=1)
                nc.gpsimd.dma_start(
                    C_bc[:], bc_dram[NSC:2 * NSC, :].partition_broadcast(128))

                # ---- selective scan per channel group --------------------
                hlast = sb.tile([128, NG, NSC], BF16, tag="hlast", bufs=2)
                ygated = sb.tile([128, NG, TC], BF16, tag="ygated", bufs=1)
                for g in range(NG):
                    h_t = sb.tile([128, NSC, TC], BF16, tag="h_t", bufs=1)
                    for n in range(NSC):
                        dA = sb.tile([128, TC], BF16, tag="dA", bufs=2)
                        nc.scalar.activation(dA[:], dt_t[:, g, :], AF.Exp,
                                             scale=A_sb[:, g, n:n + 1])
                        wv = sb.tile([128, TC], BF16, tag="wv", bufs=1)
                        nc.vector.tensor_tensor(wv[:], dtu[:, g, :],
                                                B_bc[:, n, :], Alu.mult)
                        init = 0.0 if ci == 0 else hlast_prev[:, g, n:n + 1]
                        nc.vector.tensor_tensor_scan(
                            h_t[:, n, :], dA[:], wv[:], init,
                            Alu.mult, Alu.add,
                        )
                    # save last columns for the next chunk's initial state
                    nc.vector.tensor_copy(hlast[:, g, :], h_t[:, :, TC - 1:TC])
                    hC = sb.tile([128, NSC, TC], BF16, tag="hC", bufs=1)
                    nc.vector.tensor_tensor(hC[:], h_t[:], C_bc[:], Alu.mult)
                    t1 = sb.tile([128, 4, TC], BF16, tag="t1", bufs=1)
                    nc.vector.tensor_tensor(t1[:], hC[:, 0:4, :], hC[:, 4:8, :], Alu.add)
                    t2 = sb.tile([128, 2, TC], BF16, tag="t2", bufs=1)
                    nc.vector.tensor_tensor(t2[:], t1[:, 0:2, :], t1[:, 2:4, :], Alu.add)
                    ysum = sb.tile([128, TC], BF16, tag="ysum", bufs=2)
                    nc.vector.tensor_tensor(ysum[:], t2[:, 0, :], t2[:, 1, :], Alu.add)
                    # gating: (y + D*xc) * silu(z)
                    tg = sb.tile([128, TC], BF16, tag="tg", bufs=1)
                    nc.vector.scalar_tensor_tensor(
                        tg[:], xc_t[:, g, :], D_sb[:, g, :], ysum[:],
                        Alu.mult, Alu.add,
                    )
                    nc.gpsimd.tensor_tensor(ygated[:, g, :], tg[:], gz[:, g, :],
                                            Alu.mult)
                hlast_prev = hlast

                # ---- out_proj + residual ---------------------------------
                for mo in range(MO):
                    o_ps = ps.tile([128, TC], F32, tag="o_ps", bufs=2)
                    for g in range(NG):
                        nc.tensor.matmul(
                            o_ps[:], w_out_sb[:, g, mo * 128:(mo + 1) * 128],
                            ygated[:, g, :], start=(g == 0), stop=(g == NG - 1),
                        )
                    out_sb = sb.tile([128, TC], F32, tag="out_sb", bufs=2)
                    nc.vector.scalar_tensor_tensor(
                        out_sb[:], x_res[:, mo, :], rgate_sb[:, 0:1], o_ps[:],
                        Alu.mult, Alu.add,
                    )
                    nc.sync.dma_start(outT[mo * 128:(mo + 1) * 128, ts], out_sb[:])

                xn_prev = xn

    nc.compile()
    return nc, c


# ---------------------------------------------------------------------------
# Host-side sharding
# ---------------------------------------------------------------------------

def host_shard(inputs, cfg):
    """Build the 8 per-core input maps from the full problem inputs."""
    c = derived(cfg)
    T, DM, DH, DI, DTR, NSC = c["T"], c["DM"], c["DH"], c["DI"], c["DTR"], c["NSC"]
    NKF = c["NKF"]

    x = np.asarray(inputs["x"], np.float32)          # (B, T, DM)
    norm_w = np.asarray(inputs["norm_w"], np.float32)
    norm_b = np.asarray(inputs["norm_b"], np.float32)

    in_maps = []
    for b in range(2):
        for d in range(2):
            pre = "fwd" if d == 0 else "bwd"
            if d == 0:
                xb = x[b]
                nw, nb = norm_w, norm_b
            else:
                xb = x[b][::-1]
                xb = np.concatenate([xb[:, DH:], xb[:, :DH]], axis=1)
                nw = np.concatenate([norm_w[DH:], norm_w[:DH]])
                nb = np.concatenate([norm_b[DH:], norm_b[:DH]])
            xT = np.ascontiguousarray(xb.T, dtype=np.float32)

            W = np.asarray(inputs[pre + "_in_proj_w"], np.float32)   # (2DI, DH)
            conv_w = np.asarray(inputs[pre + "_conv_w"], np.float32)[:, 0, :]
            conv_b = np.asarray(inputs[pre + "_conv_b"], np.float32)
            xp = np.asarray(inputs[pre + "_x_proj_w"], np.float32)
            wdt = np.asarray(inputs[pre + "_dt_proj_w"], np.float32)
            dtb = np.asarray(inputs[pre + "_dt_proj_b"], np.float32)
            A_log = np.asarray(inputs[pre + "_A_log"], np.float32)
            Dv = np.asarray(inputs[pre + "_D"], np.float32)
            wout = np.asarray(inputs[pre + "_out_proj_w"], np.float32)

            nwh, nbh = nw[:DH], nb[:DH]
            W_eff = W * nwh[None, :]
            bias_in = W @ nbh                                        # (2DI,)
            W_xc, W_z = W_eff[:DI], W_eff[DI:]

            blocks = []
            for j in range(4):
                scaled = conv_w[:, j:j + 1].T * W_xc.T               # (DH, DI)
                for kk in range(NKF):
                    blocks.append(scaled[kk * 128:(kk + 1) * 128, :])
            w_xc_T = np.ascontiguousarray(np.concatenate(blocks, 0)).astype(BF)
            bias_xc = (conv_b + bias_in[:DI] * conv_w.sum(1)).reshape(DI, 1)
            w_z_T = np.ascontiguousarray(W_z.T).astype(BF)
            bias_z = bias_in[DI:].reshape(DI, 1).astype(np.float32)

            base = dict(
                xT=xT,
                w_xc_T=w_xc_T,
                w_z_T=w_z_T,
                w_dt_T=np.ascontiguousarray(wdt.T).astype(BF),
                w_out_T=np.ascontiguousarray(wout.T).astype(BF),
                bias_xc=bias_xc.astype(np.float32),
                bias_z=bias_z,
                dt_bias=dtb.reshape(DI, 1).astype(np.float32),
            )
            for nh in range(2):
                sel = np.concatenate([
                    xp[:DTR],
                    xp[DTR + NSC * nh: DTR + NSC * nh + NSC],
                    xp[DTR + D_STATE + NSC * nh: DTR + D_STATE + NSC * nh + NSC],
                ], axis=0)                                           # (DTR+16, DI)
                m = dict(base)
                m["w_xp_T"] = np.ascontiguousarray(sel.T).astype(BF)
                m["A_cols"] = np.ascontiguousarray(
                    -np.exp(A_log[:, NSC * nh: NSC * nh + NSC])).astype(np.float32)
                m["D_vec"] = (Dv if nh == 0 else np.zeros_like(Dv)).reshape(DI, 1).astype(np.float32)
                m["res_gate"] = np.full((128, 1), 1.0 if nh == 0 else 0.0, np.float32)
                in_maps.append(m)
    return in_maps


def host_unshard(results, cfg):
    c = derived(cfg)
    T, DM, DH = c["T"], c["DM"], c["DH"]
    out = np.empty((2, T, DM), np.float32)
    for b in range(2):
        for d in range(2):
            o = results[b * 4 + d * 2 + 0]["outT"] + results[b * 4 + d * 2 + 1]["outT"]
            oT = o.T                                   # (T, DH)
            if d == 1:
                oT = oT[::-1]
            out[b, :, d * DH:(d + 1) * DH] = oT
    return out


_CACHE = {}


def _get_nc(cfg_key):
    if cfg_key not in _CACHE:
        cfg = dict(T=cfg_key[0], DM=cfg_key[1], TC=cfg_key[2])
        _CACHE[cfg_key] = build_nc(cfg)
    return _CACHE[cfg_key]


def kernel(**inputs):
    cfg = default_cfg()
    nc, _ = _get_nc((cfg["T"], cfg["DM"], cfg["TC"]))
    in_maps = host_shard(inputs, cfg)
    res = bass_utils.run_bass_kernel_spmd(nc, in_maps, core_ids=list(range(8)))
    return host_unshard(res.results, cfg)
